# revision 1
# baseline (speedup 1.0000x reference)
"""Trainium2 Bass kernel for the DIN embedding-lookup model.

Strategy (data-parallel over 8 NeuronCores, batch sharded):
  - Each core handles 2048 samples (16 tiles of 128 samples).
  - Embedding tables stay in HBM; rows are fetched with indirect (gather)
    DMA: one descriptor per 40-byte row, ~25.6K rows per instruction.
  - History pooling (mean over 200 rows) is a strided DVE reduce; the
    1/200 scaling is folded into the first MLP weight matrix on the host.
  - The per-sample feature vector tiles [128, 40] are transposed on the
    tensor engine into a feature-major activation [40, 2048], so the tiny
    MLP runs as three PE matmuls with fused bias+ReLU / bias+Sigmoid on
    the scalar engine.  softmax(logits)[:, 1] == sigmoid(l1 - l0), so the
    last layer collapses to a single output column.
"""

import numpy as np

B, L, K = 16384, 200, 10
N_USER, N_MOVIE, N_CATE = 138500, 131270, 138500
NCORES = 8
BS = B // NCORES          # samples per core
P = 128                   # partitions / samples per tile
NT = BS // P              # tiles per core
H1, H2 = 20, 8            # MLP widths
NCHUNK = 512              # MLP free-dim chunk

_CACHE = {}

# 0: ACT Sigmoid LUT; 1: exp+reciprocal; 2: also emit pre-sigmoid logits
SIGMOID_MODE = 1
DEBUG_Z = False
# "column": HW-validated one-index-per-partition gathers (128 rows/inst).
# "block": one [128,L,1]-offset gather per tile-history (25600 rows/inst) —
#          only valid if the 3-dim offset AP probe (gathertest4) passes.
GATHER_MODE = "column"


def _build(reps=1):
    key = ("nc", SIGMOID_MODE, DEBUG_Z, reps, GATHER_MODE)
    if key in _CACHE:
        return _CACHE[key]

    import concourse.bass as bass
    import concourse.tile as tile
    from concourse import bacc, mybir
    from concourse.masks import make_identity

    f32 = mybir.dt.float32
    i32 = mybir.dt.int32
    AF = mybir.ActivationFunctionType

    nc = bacc.Bacc(
        "TRN2", target_bir_lowering=False, debug=False, num_devices=NCORES
    )

    uid = nc.dram_tensor("uid_idx", [P, NT], i32, kind="ExternalInput").ap()
    mid = nc.dram_tensor("mid_idx", [P, NT], i32, kind="ExternalInput").ap()
    mc = nc.dram_tensor("mc_idx", [P, NT, L], i32, kind="ExternalInput").ap()
    ur = nc.dram_tensor("ur_idx", [P, NT, L], i32, kind="ExternalInput").ap()
    ue = nc.dram_tensor("user_emb", [N_USER, K], f32, kind="ExternalInput").ap()
    me = nc.dram_tensor("movie_emb", [N_MOVIE, K], f32, kind="ExternalInput").ap()
    ce = nc.dram_tensor("cate_emb", [N_CATE, K], f32, kind="ExternalInput").ap()
    w1t = nc.dram_tensor("w1t", [4 * K, H1], f32, kind="ExternalInput").ap()
    b1 = nc.dram_tensor("b1", [H1, 1], f32, kind="ExternalInput").ap()
    w2t = nc.dram_tensor("w2t", [H1, H2], f32, kind="ExternalInput").ap()
    b2 = nc.dram_tensor("b2", [H2, 1], f32, kind="ExternalInput").ap()
    w3 = nc.dram_tensor("w3", [H2, 1], f32, kind="ExternalInput").ap()
    b3 = nc.dram_tensor("b3", [1, 1], f32, kind="ExternalInput").ap()
    out = nc.dram_tensor("out", [BS], f32, kind="ExternalOutput").ap()
    out_z = (
        nc.dram_tensor("out_z", [BS], f32, kind="ExternalOutput").ap()
        if DEBUG_Z
        else None
    )

    with tile.TileContext(nc) as tc:
        with (
            tc.tile_pool(name="singles", bufs=1) as singles,
            tc.tile_pool(name="gath", bufs=3) as gpool,
            tc.tile_pool(name="fea", bufs=3) as fpool,
            tc.tile_pool(name="mlp", bufs=2) as mpool,
            tc.tile_pool(name="ptr", bufs=2, space="PSUM") as ptr,
            tc.tile_pool(name="pmlp", bufs=2, space="PSUM") as pmlp,
        ):
            ident = singles.tile([P, P], f32)
            make_identity(nc, ident[:])

            uid_sb = singles.tile([P, NT], i32)
            nc.sync.dma_start(out=uid_sb[:], in_=uid[:])
            mid_sb = singles.tile([P, NT], i32)
            nc.sync.dma_start(out=mid_sb[:], in_=mid[:])
            mc_sb = singles.tile([P, NT, L], i32)
            nc.sync.dma_start(out=mc_sb[:], in_=mc[:])
            ur_sb = singles.tile([P, NT, L], i32)
            nc.sync.dma_start(out=ur_sb[:], in_=ur[:])

            w1t_sb = singles.tile([4 * K, H1], f32)
            nc.sync.dma_start(out=w1t_sb[:], in_=w1t[:])
            b1_sb = singles.tile([H1, 1], f32)
            nc.sync.dma_start(out=b1_sb[:], in_=b1[:])
            w2t_sb = singles.tile([H1, H2], f32)
            nc.sync.dma_start(out=w2t_sb[:], in_=w2t[:])
            b2_sb = singles.tile([H2, 1], f32)
            nc.sync.dma_start(out=b2_sb[:], in_=b2[:])
            w3_sb = singles.tile([H2, 1], f32)
            nc.sync.dma_start(out=w3_sb[:], in_=w3[:])
            b3_sb = singles.tile([1, 1], f32)
            nc.sync.dma_start(out=b3_sb[:], in_=b3[:])
            nb3_sb = singles.tile([1, 1], f32)
            nc.scalar.mul(nb3_sb[:], b3_sb[:], -1.0)

            # Gather a = user_emb[user_id], b = movie_emb[movie_id]
            # (one [128,1]->[128,K] indirect DMA per tile column)
            a_sb = singles.tile([P, NT, K], f32)
            for t in range(NT):
                nc.gpsimd.indirect_dma_start(
                    out=a_sb[:, t, :],
                    out_offset=None,
                    in_=ue[:],
                    in_offset=bass.IndirectOffsetOnAxis(
                        ap=uid_sb[:, t : t + 1], axis=0
                    ),
                )
            bm_sb = singles.tile([P, NT, K], f32)
            for t in range(NT):
                nc.gpsimd.indirect_dma_start(
                    out=bm_sb[:, t, :],
                    out_offset=None,
                    in_=me[:],
                    in_offset=bass.IndirectOffsetOnAxis(
                        ap=mid_sb[:, t : t + 1], axis=0
                    ),
                )

            feaT = singles.tile([4 * K, NT * P], f32)

            for _rep in range(reps):
                _loop_body(
                    nc, bass, mybir, gpool, fpool, mpool, ptr, pmlp, singles,
                    ident, uid_sb, mid_sb, mc_sb, ur_sb, a_sb, bm_sb, feaT,
                    w1t_sb, b1_sb, w2t_sb, b2_sb, w3_sb, b3_sb, nb3_sb,
                    ue, me, ce, out, out_z,
                )

    nc.compile()
    _CACHE[key] = nc
    return nc


def _loop_body(
    nc, bass, mybir, gpool, fpool, mpool, ptr, pmlp, singles,
    ident, uid_sb, mid_sb, mc_sb, ur_sb, a_sb, bm_sb, feaT,
    w1t_sb, b1_sb, w2t_sb, b2_sb, w3_sb, b3_sb, nb3_sb,
    ue, me, ce, out, out_z,
):
    f32 = mybir.dt.float32
    AF = mybir.ActivationFunctionType
    if True:
        if True:
            for t in range(NT):
                # HW indirect-DMA contract: ONE index per partition per
                # instruction (idx [128,1] -> dest [128,K]), same as the
                # XLA gather lowering.  200 column gathers per history.
                g_c = gpool.tile([P, L, K], f32, tag="g_c")
                g_d = gpool.tile([P, L, K], f32, tag="g_d")
                if GATHER_MODE == "column":
                    for j in range(L):
                        nc.gpsimd.indirect_dma_start(
                            out=g_c[:, j, :],
                            out_offset=None,
                            in_=ce[:],
                            in_offset=bass.IndirectOffsetOnAxis(
                                ap=mc_sb[:, t, j : j + 1], axis=0
                            ),
                        )
                    for j in range(L):
                        nc.gpsimd.indirect_dma_start(
                            out=g_d[:, j, :],
                            out_offset=None,
                            in_=ce[:],
                            in_offset=bass.IndirectOffsetOnAxis(
                                ap=ur_sb[:, t, j : j + 1], axis=0
                            ),
                        )
                else:
                    nc.gpsimd.indirect_dma_start(
                        out=g_c[:],
                        out_offset=None,
                        in_=ce[:],
                        in_offset=bass.IndirectOffsetOnAxis(
                            ap=mc_sb[:, t, :].rearrange("p (l o) -> p l o", o=1),
                            axis=0,
                        ),
                    )
                    nc.gpsimd.indirect_dma_start(
                        out=g_d[:],
                        out_offset=None,
                        in_=ce[:],
                        in_offset=bass.IndirectOffsetOnAxis(
                            ap=ur_sb[:, t, :].rearrange("p (l o) -> p l o", o=1),
                            axis=0,
                        ),
                    )

                fea_t = fpool.tile([P, 4 * K], f32, tag="fea_t")
                nc.vector.tensor_copy(fea_t[:, 0:K], a_sb[:, t, :])
                nc.vector.tensor_copy(fea_t[:, K : 2 * K], bm_sb[:, t, :])
                nc.vector.tensor_reduce(
                    out=fea_t[:, 2 * K : 3 * K],
                    in_=g_c[:].rearrange("p l k -> p k l"),
                    axis=mybir.AxisListType.X,
                    op=mybir.AluOpType.add,
                )
                nc.vector.tensor_reduce(
                    out=fea_t[:, 3 * K : 4 * K],
                    in_=g_d[:].rearrange("p l k -> p k l"),
                    axis=mybir.AxisListType.X,
                    op=mybir.AluOpType.add,
                )

                tr = ptr.tile([4 * K, P], f32, tag="tr")
                nc.tensor.transpose(out=tr[:], in_=fea_t[:], identity=ident[:])
                nc.scalar.copy(out=feaT[:, t * P : (t + 1) * P], in_=tr[:])

            for n in range(BS // NCHUNK):
                sl = slice(n * NCHUNK, (n + 1) * NCHUNK)
                p1 = pmlp.tile([H1, NCHUNK], f32, tag="p1")
                nc.tensor.matmul(
                    p1[:], lhsT=w1t_sb[:], rhs=feaT[:, sl], start=True, stop=True
                )
                h1 = mpool.tile([H1, NCHUNK], f32, tag="h1")
                nc.scalar.activation(h1[:], p1[:], AF.Relu, bias=b1_sb[:])
                p2 = pmlp.tile([H2, NCHUNK], f32, tag="p2")
                nc.tensor.matmul(
                    p2[:], lhsT=w2t_sb[:], rhs=h1[:], start=True, stop=True
                )
                h2 = mpool.tile([H2, NCHUNK], f32, tag="h2")
                nc.scalar.activation(h2[:], p2[:], AF.Relu, bias=b2_sb[:])
                p3 = pmlp.tile([1, NCHUNK], f32, tag="p3")
                nc.tensor.matmul(
                    p3[:], lhsT=w3_sb[:], rhs=h2[:], start=True, stop=True
                )
                if DEBUG_Z:
                    zraw = mpool.tile([1, NCHUNK], f32, tag="zraw")
                    nc.vector.tensor_scalar_add(zraw[:], p3[:], b3_sb[:])
                    nc.sync.dma_start(out=out_z[sl], in_=zraw[:])
                z = mpool.tile([1, NCHUNK], f32, tag="z")
                if SIGMOID_MODE == 0:
                    nc.scalar.activation(z[:], p3[:], AF.Sigmoid, bias=b3_sb[:])
                else:
                    # p = 1 / (1 + exp(-(z+b3))):  ACT Exp with scale=-1,
                    # then +1 and an exact DVE reciprocal.
                    e = mpool.tile([1, NCHUNK], f32, tag="e")
                    nc.scalar.activation(
                        e[:], p3[:], AF.Exp, bias=nb3_sb[:], scale=-1.0
                    )
                    nc.vector.tensor_scalar_add(e[:], e[:], 1.0)
                    nc.vector.reciprocal(z[:], e[:])
                nc.sync.dma_start(out=out[sl], in_=z[:])


def _prep_core_inputs(c, uid, mid, mc, ur, ue, me, ce, w1t, b1, w2t, b2, w3, b3):
    sl = slice(c * BS, (c + 1) * BS)
    return {
        "uid_idx": np.ascontiguousarray(uid[sl].reshape(NT, P).T),
        "mid_idx": np.ascontiguousarray(mid[sl].reshape(NT, P).T),
        "mc_idx": np.ascontiguousarray(mc[sl].reshape(NT, P, L).transpose(1, 0, 2)),
        "ur_idx": np.ascontiguousarray(ur[sl].reshape(NT, P, L).transpose(1, 0, 2)),
        "user_emb": ue,
        "movie_emb": me,
        "cate_emb": ce,
        "w1t": w1t,
        "b1": b1,
        "w2t": w2t,
        "b2": b2,
        "w3": w3,
        "b3": b3,
    }


def kernel(
    user_id,
    movie_id,
    movie_cate,
    user_rate,
    user_emb,
    movie_emb,
    cate_emb,
    W1,
    b1,
    W2,
    b2,
    W3,
    b3,
):
    from concourse.bass_utils import run_bass_kernel_spmd

    uid = np.asarray(user_id).astype(np.int32, copy=False)
    mid = np.asarray(movie_id).astype(np.int32, copy=False)
    mc = np.asarray(movie_cate).astype(np.int32, copy=False)
    ur = np.asarray(user_rate).astype(np.int32, copy=False)
    ue = np.ascontiguousarray(np.asarray(user_emb, dtype=np.float32))
    me = np.ascontiguousarray(np.asarray(movie_emb, dtype=np.float32))
    ce = np.ascontiguousarray(np.asarray(cate_emb, dtype=np.float32))
    W1 = np.asarray(W1, dtype=np.float32)
    W2 = np.asarray(W2, dtype=np.float32)
    W3 = np.asarray(W3, dtype=np.float32)
    b1 = np.asarray(b1, dtype=np.float32)
    b2 = np.asarray(b2, dtype=np.float32)
    b3 = np.asarray(b3, dtype=np.float32)

    # Fold the 1/L mean scaling into W1's pooled-feature columns, and the
    # 2-way softmax into a single sigmoid column (p1 = sigmoid(l1 - l0)).
    W1s = W1.copy()
    W1s[:, 2 * K :] *= 1.0 / L
    w1t = np.ascontiguousarray(W1s.T)                      # [40, 20]
    b1c = np.ascontiguousarray(b1.reshape(H1, 1))
    w2t = np.ascontiguousarray(W2.T)                       # [20, 8]
    b2c = np.ascontiguousarray(b2.reshape(H2, 1))
    w3c = np.ascontiguousarray((W3[1] - W3[0]).reshape(H2, 1))
    b3c = np.array([[b3[1] - b3[0]]], dtype=np.float32)

    nc = _build()
    in_maps = [
        _prep_core_inputs(c, uid, mid, mc, ur, ue, me, ce, w1t, b1c, w2t, b2c, w3c, b3c)
        for c in range(NCORES)
    ]
    res = run_bass_kernel_spmd(nc, in_maps, core_ids=list(range(NCORES)))
    return np.concatenate([res.results[c]["out"] for c in range(NCORES)])

